# revision 13
# baseline (speedup 1.0000x reference)
"""Causal depthwise conv1d (K=3) + pointwise 1x1 conv for Trainium2.

Full-input contract: kernel(**inputs) takes the complete (unsharded) numpy
inputs and returns the complete output. Internally the work is sharded over
8 NeuronCores: core c handles batch b = c//2 and sequence half c%2
(L_chunk = 2048), with a (K-1)=2 column halo taken from the previous
sequence chunk (zeros at the causal left edge). The small conv weights are
replicated on every core.

All device I/O uses partition-major, tile-major contiguous DRAM layouts so
every DMA lowers to 128 large (4-24 KB) descriptors -- strided per-row
descriptors (~0.5-1 KB) measured 4-8x slower end-to-end.

Per-core compute layout is channel-major ([P=128 partitions, DC=8 chunks,
cols]). The depthwise conv per chunk: tap0 (+b_dw) on the scalar engine
(activation), taps 1+2 accumulate in-place into a bf16 y tile via DVE
scalar_tensor_tensor (STT has no packed 2x uop on trn2 -> ~1.04 ns/col
regardless of dtype, so the split ACT/DVE balances the two engines under
the PE's tile period). The pointwise conv is a bf16 K-contraction matmul;
PSUM is evacuated by ACT (fused + b_pw, bf16 out).

Schedule notes (from perfetto traces):
  - ~7us fixed framework preamble; first DMA issues right after it.
  - PE HAM clock gate: cold 1.2 GHz until ~3.4us of sustained matmul
    activity. A burst of dummy matmuls right after the preamble warms the
    PE while the first x tile + dw conv are still in flight.
  - Small first/last l-tiles shrink the pipeline fill/drain.
"""

import sys

if "/opt/trn_rl_repo" not in sys.path:
    sys.path.insert(0, "/opt/trn_rl_repo")

import numpy as np

import concourse.bass as bass
import concourse.tile as tile
from concourse import bacc, mybir
from concourse.bass_utils import run_bass_kernel_spmd

P = 128          # SBUF partitions
B, L, D = 4, 4096, 1024
KSZ = 3          # depthwise kernel taps
NCORES = 8
LC = (B * L) // NCORES   # 2048 sequence positions per core
PAD = 4          # 2 junk + 2 halo columns per chunk row (4B row alignment)
# l-tile schedule: small first tile shrinks the serial prologue (PE starts
# on the first columns early), small last tile shrinks the store drain.
LTS = [256, 512, 512, 512, 256]
assert sum(LTS) == LC
DC = D // P              # 8 channel chunks (contraction)
EC = D // P              # 8 output-channel chunks

NDUMMY = 10      # PE pre-warm matmuls (HAM un-throttle before first real MM)

# chunks where tap1 is offloaded: ACT makes t1 = w1*x1 (alignment doesn't
# matter on the scalar engine), then GpSimd adds it into y with a plain
# tensor_tensor. DVE ops with per-partition scalars are 1x whenever any
# tensor operand is 2B-misaligned (and scalar_tensor_tensor is 1x always),
# so this is the only way to shed DVE work. Keeps every engine's per-tile
# time strictly under the PE's so the pipeline never starves the PE.
GPS_CHUNKS = (2, 4, 6)
# e-chunks whose PSUM evacuation runs on DVE (tensor_scalar +b_pw) instead
# of ACT, balancing the two engines
DVE_EVAC = (0, 1)

_CACHED_NC = None

_LT_OFF = [0]
for _n in LTS:
    _LT_OFF.append(_LT_OFF[-1] + _n)
# flat column offset of each tile block in the packed x dram tensor
_XBLK = [0]
for _n in LTS:
    _XBLK.append(_XBLK[-1] + DC * (_n + PAD))


def _build_nc():
    nc = bacc.Bacc("TRN2", target_bir_lowering=False, debug=False,
                   num_devices=NCORES)
    f32 = mybir.dt.float32
    bf16 = mybir.dt.bfloat16

    # x packed per partition, tile-major: block lt = [DC, n+PAD] per
    # partition, cols 2..3 = halo, 4.. = data (tap k reads cols 2+k..)
    xt = nc.dram_tensor("xt", [P, _XBLK[-1]], bf16, kind="ExternalInput").ap()
    # weights packed per partition: wt[p, ec*1024 + dc*128 + j]
    #   = w_pw[ec*128+j, dc*128+p]
    wt = nc.dram_tensor("wt", [P, EC * DC * P], bf16,
                        kind="ExternalInput").ap()
    # per-channel params fp32, cols: w_dw[0..2], b_dw, b_pw
    pp = nc.dram_tensor("pp", [D, 5], f32, kind="ExternalInput").ap()
    # output packed per partition, tile-major: block lt = [EC, n]
    ot = nc.dram_tensor("ot", [P, EC * LC], bf16, kind="ExternalOutput").ap()

    pp_r = pp.rearrange("(o p) c -> p o c", p=P)    # [128, DC, 5]

    with tile.TileContext(nc) as tc:
        with (
            tc.tile_pool(name="wpool", bufs=1) as wpool,
            tc.tile_pool(name="ppool", bufs=1) as ppool,
            tc.tile_pool(name="dpool", bufs=1) as dpool,
            tc.tile_pool(name="xpool5", bufs=3) as xpool5,
            tc.tile_pool(name="xpool2", bufs=2) as xpool2,
            tc.tile_pool(name="ypool", bufs=18) as ypool,
            tc.tile_pool(name="tpool", bufs=6) as tpool,
            tc.tile_pool(name="opool5", bufs=2) as opool5,
            tc.tile_pool(name="opool2", bufs=2) as opool2,
            tc.tile_pool(name="psum", bufs=7, space="PSUM") as psum_pool,
            tc.tile_pool(name="dpsum", bufs=1, space="PSUM") as dpsum_pool,
        ):
            p_sb = ppool.tile([P, DC, 5], f32)
            w_sb0 = wpool.tile([P, 1 * DC * P], bf16, name="w_sb0")
            w_sb13 = wpool.tile([P, 3 * DC * P], bf16, name="w_sb13")
            w_sb47 = wpool.tile([P, 4 * DC * P], bf16, name="w_sb47")
            dummy_rhs = dpool.tile([P, 512], bf16, name="dummy_rhs")
            dummy_act = dpool.tile([P, 8], bf16, name="dummy_act")
            dummy_ps = dpsum_pool.tile([P, 512], f32, name="dummy_ps")

            def w_ap(ec, dc):
                if ec == 0:
                    return w_sb0[:, dc * P:(dc + 1) * P]
                if ec < 4:
                    return w_sb13[:, (ec - 1) * DC * P + dc * P:][:, :P]
                return w_sb47[:, (ec - 4) * DC * P + dc * P:][:, :P]

            def x_load(lt):
                """one contiguous DMA (SP queue) for l-tile lt"""
                n = LTS[lt]
                pool = xpool5 if n == 512 else xpool2
                xs = pool.tile([P, DC, n + PAD], bf16, tag=f"x{n}",
                               name="xs")
                nc.sync.dma_start(
                    xs[:],
                    xt[:, _XBLK[lt]:_XBLK[lt + 1]].rearrange(
                        "p (o c) -> p o c", c=n + PAD))
                return xs

            def dw_conv(lt, xs):
                """depthwise conv: tap0 on ACT, taps 1+2 in-place on DVE;
                on GPS_CHUNKS tap1 rides DVE-TS + GpSimd-TT instead"""
                n = LTS[lt]
                ys = []
                for dc in range(DC):
                    y = ypool.tile([P, 512], bf16, tag="y", name="y")[:, :n]
                    gps = dc in GPS_CHUNKS
                    nc.scalar.activation(
                        y[:], xs[:, dc, 2:2 + n],
                        mybir.ActivationFunctionType.Identity,
                        bias=p_sb[:, dc, 3:4], scale=p_sb[:, dc, 0:1])
                    if gps:
                        t1 = tpool.tile([P, 512], bf16, tag="t1",
                                        name="t1")[:, :n]
                        nc.scalar.activation(
                            t1[:], xs[:, dc, 3:3 + n],
                            mybir.ActivationFunctionType.Identity,
                            bias=0.0, scale=p_sb[:, dc, 1:2])
                    if not gps:
                        nc.vector.scalar_tensor_tensor(
                            y[:], xs[:, dc, 3:3 + n], p_sb[:, dc, 1:2], y[:],
                            op0=mybir.AluOpType.mult,
                            op1=mybir.AluOpType.add)
                    nc.vector.scalar_tensor_tensor(
                        y[:], xs[:, dc, 4:4 + n], p_sb[:, dc, 2:3], y[:],
                        op0=mybir.AluOpType.mult, op1=mybir.AluOpType.add)
                    if gps:
                        nc.gpsimd.tensor_tensor(
                            y[:], y[:], t1[:], op=mybir.AluOpType.add)
                    ys.append(y)
                return ys

            def pointwise(lt, ys, o_sb, ecs):
                """o_sb[:, ec, :] = w_pw[ec] @ y + b_pw[ec] for e-chunks"""
                n = LTS[lt]
                for ec in ecs:
                    acc = psum_pool.tile([P, 512], f32, tag="acc",
                                         name="acc")[:, :n]
                    for dc in range(DC):
                        nc.tensor.matmul(
                            acc[:],
                            lhsT=w_ap(ec, dc),
                            rhs=ys[dc][:],
                            start=(dc == 0), stop=(dc == DC - 1))
                    if ec in DVE_EVAC:
                        nc.vector.tensor_scalar(
                            o_sb[:, ec, :], acc[:],
                            p_sb[:, ec, 4:5], 0.0,
                            op0=mybir.AluOpType.add,
                            op1=mybir.AluOpType.add)
                    else:
                        nc.scalar.activation(
                            o_sb[:, ec, :], acc[:],
                            mybir.ActivationFunctionType.Identity,
                            bias=p_sb[:, ec, 4:5], scale=1.0)

            def o_tile(lt):
                n = LTS[lt]
                pool = opool5 if n == 512 else opool2
                return pool.tile([P, EC, n], bf16, tag=f"o{n}", name="o_sb")

            def store(lt, o_sb):
                n = LTS[lt]
                s = _LT_OFF[lt]
                # alternate queues so no single DGE ring backs up
                eng = nc.gpsimd if lt % 2 == 0 else nc.sync
                eng.dma_start(
                    ot[:, EC * s:EC * (s + n)].rearrange(
                        "p (e c) -> p e c", c=n),
                    o_sb[:])

            # --- emission (guides per-queue FIFO order) -----------------
            # warm-up: DVE memsets a junk rhs, ACT preloads its table, the
            # PE chews dummy matmuls so HAM un-throttles during the DMA wait
            nc.vector.memset(dummy_rhs[:], 0.0)
            nc.scalar.activation(
                dummy_act[:], dummy_rhs[:, 0:8],
                mybir.ActivationFunctionType.Identity, bias=0.0, scale=1.0)

            # x rides the SP HWDGE queue; params + weights ride the
            # (otherwise idle early) GpSimd SWDGE queue
            xs0 = x_load(0)
            nc.gpsimd.dma_start(p_sb[:], pp_r[:])
            nc.gpsimd.dma_start(w_sb0[:], wt[:, 0:DC * P])

            for _ in range(NDUMMY):
                nc.tensor.matmul(dummy_ps[:], lhsT=dummy_rhs[:, 0:P],
                                 rhs=dummy_rhs[:], start=True, stop=True)

            nc.gpsimd.dma_start(w_sb13[:], wt[:, DC * P:4 * DC * P])
            xs1 = x_load(1)
            ys0 = dw_conv(0, xs0)
            nc.gpsimd.dma_start(w_sb47[:], wt[:, 4 * DC * P:8 * DC * P])
            o0 = o_tile(0)
            pointwise(0, ys0, o0, range(0, 4))
            xs2 = x_load(2)
            ys1 = dw_conv(1, xs1)
            pointwise(0, ys0, o0, range(4, EC))
            store(0, o0)

            xs3 = x_load(3)
            ys2 = dw_conv(2, xs2)
            o1 = o_tile(1)
            pointwise(1, ys1, o1, range(EC))
            store(1, o1)

            xs4 = x_load(4)
            ys3 = dw_conv(3, xs3)
            o2 = o_tile(2)
            pointwise(2, ys2, o2, range(EC))
            store(2, o2)

            ys4 = dw_conv(4, xs4)
            o3 = o_tile(3)
            pointwise(3, ys3, o3, range(EC))
            store(3, o3)

            # last tile: store per ec-pair, spread across the three DMA
            # queues so the final drain is short
            n4 = LTS[4]
            s4 = _LT_OFF[4]
            o4 = o_tile(4)
            last_q = [nc.sync, nc.gpsimd, nc.scalar, nc.sync]
            for i in range(4):
                pointwise(4, ys4, o4, range(2 * i, 2 * i + 2))
                dst = ot[:, EC * s4 + 2 * i * n4:EC * s4 + (2 * i + 2) * n4]
                last_q[i].dma_start(
                    dst.rearrange("p (e c) -> p e c", c=n4),
                    o4[:, 2 * i:2 * i + 2, :])

    nc.compile()  # bacc: legalizes multi-sem waits for TRN2 codegen
    return nc


def _shard_inputs(x, w_dw, b_dw, w_pw, b_pw):
    import ml_dtypes
    bf = ml_dtypes.bfloat16
    # wt[p, ec*1024 + dc*128 + j] = w_pw[ec*128+j, dc*128+p]
    wt = np.ascontiguousarray(
        w_pw.reshape(EC, P, DC, P).transpose(3, 0, 2, 1).reshape(P, -1)
    ).astype(bf)
    pp = np.ascontiguousarray(
        np.stack([w_dw[:, 0], w_dw[:, 1], w_dw[:, 2], b_dw, b_pw], axis=1),
        dtype=np.float32)                                        # (D, 5)
    in_maps = []
    for c in range(NCORES):
        b, half = divmod(c, 2)
        l0 = half * LC
        # xpad[d, t]: t 0..1 junk, 2..3 halo (x[l0-2], x[l0-1]), 4.. data
        xpad = np.zeros((D, LC + PAD), dtype=bf)
        lo = max(l0 - 2, 0)
        xpad[:, PAD - (l0 - lo):] = x[b, lo:l0 + LC, :].T.astype(bf)
        xtc = np.empty((P, _XBLK[-1]), dtype=bf)
        for lt, n in enumerate(LTS):
            s = _LT_OFF[lt]
            blk = xpad[:, s:s + n + PAD].reshape(DC, P, n + PAD)
            xtc[:, _XBLK[lt]:_XBLK[lt + 1]] = \
                blk.transpose(1, 0, 2).reshape(P, -1)
        in_maps.append({"xt": xtc, "wt": wt, "pp": pp})
    return in_maps


def kernel(x, w_dw, b_dw, w_pw, b_pw):
    assert x.shape == (B, L, D) and w_dw.shape == (D, KSZ)
    global _CACHED_NC
    if _CACHED_NC is None:
        _CACHED_NC = _build_nc()
    in_maps = _shard_inputs(np.asarray(x, dtype=np.float32),
                            np.asarray(w_dw), np.asarray(b_dw),
                            np.asarray(w_pw), np.asarray(b_pw))
    results = run_bass_kernel_spmd(
        _CACHED_NC, in_maps, list(range(NCORES))).results
    out = np.empty((B, L, D), dtype=np.float32)
    for c in range(NCORES):
        b, half = divmod(c, 2)
        l0 = half * LC
        o = results[c]["ot"]
        for lt, n in enumerate(LTS):
            s = _LT_OFF[lt]
            blk = o[:, EC * s:EC * (s + n)].reshape(P, EC, n)
            out[b, l0 + s:l0 + s + n, :] = \
                blk.transpose(2, 1, 0).reshape(n, D).astype(np.float32)
    return out


# revision 14
# speedup vs baseline: 1.0329x; 1.0329x over previous
"""Causal depthwise conv1d (K=3) + pointwise 1x1 conv for Trainium2.

Full-input contract: kernel(**inputs) takes the complete (unsharded) numpy
inputs and returns the complete output. Internally the work is sharded over
8 NeuronCores: core c handles batch b = c//2 and sequence half c%2
(L_chunk = 2048), with a (K-1)=2 column halo taken from the previous
sequence chunk (zeros at the causal left edge). The small conv weights are
replicated on every core.

All device I/O uses partition-major, tile-major contiguous DRAM layouts so
every DMA lowers to 128 large (4-24 KB) descriptors -- strided per-row
descriptors (~0.5-1 KB) measured 4-8x slower end-to-end.

Per-core compute layout is channel-major ([P=128 partitions, DC=8 chunks,
cols]). The depthwise conv per chunk: tap0 (+b_dw) on the scalar engine
(activation), taps 1+2 accumulate in-place into a bf16 y tile via DVE
scalar_tensor_tensor (STT has no packed 2x uop on trn2 -> ~1.04 ns/col
regardless of dtype, so the split ACT/DVE balances the two engines under
the PE's tile period). The pointwise conv is a bf16 K-contraction matmul;
PSUM is evacuated by ACT (fused + b_pw, bf16 out).

Schedule notes (from perfetto traces):
  - ~7us fixed framework preamble; first DMA issues right after it.
  - PE HAM clock gate: cold 1.2 GHz until ~3.4us of sustained matmul
    activity. A burst of dummy matmuls right after the preamble warms the
    PE while the first x tile + dw conv are still in flight.
  - Small first/last l-tiles shrink the pipeline fill/drain.
"""

import sys

if "/opt/trn_rl_repo" not in sys.path:
    sys.path.insert(0, "/opt/trn_rl_repo")

import numpy as np

import concourse.bass as bass
import concourse.tile as tile
from concourse import bacc, mybir
from concourse.bass_utils import run_bass_kernel_spmd

P = 128          # SBUF partitions
B, L, D = 4, 4096, 1024
KSZ = 3          # depthwise kernel taps
NCORES = 8
LC = (B * L) // NCORES   # 2048 sequence positions per core
PAD = 4          # 2 junk + 2 halo columns per chunk row (4B row alignment)
# l-tile schedule: small first tile shrinks the serial prologue (PE starts
# on the first columns early), small last tile shrinks the store drain.
LTS = [256, 512, 512, 512, 256]
assert sum(LTS) == LC
DC = D // P              # 8 channel chunks (contraction)
EC = D // P              # 8 output-channel chunks

NDUMMY = 10      # PE pre-warm matmuls (HAM un-throttle before first real MM)

# chunks where tap1 is offloaded: ACT makes t1 = w1*x1 (alignment doesn't
# matter on the scalar engine), then GpSimd adds it into y with a plain
# tensor_tensor. DVE ops with per-partition scalars are 1x whenever any
# tensor operand is 2B-misaligned (and scalar_tensor_tensor is 1x always),
# so this is the only way to shed DVE work. Keeps every engine's per-tile
# time strictly under the PE's so the pipeline never starves the PE.
GPS_CHUNKS = (3, 6)

_CACHED_NC = None

_LT_OFF = [0]
for _n in LTS:
    _LT_OFF.append(_LT_OFF[-1] + _n)
# flat column offset of each tile block in the packed x dram tensor
_XBLK = [0]
for _n in LTS:
    _XBLK.append(_XBLK[-1] + DC * (_n + PAD))


def _build_nc():
    nc = bacc.Bacc("TRN2", target_bir_lowering=False, debug=False,
                   num_devices=NCORES)
    f32 = mybir.dt.float32
    bf16 = mybir.dt.bfloat16

    # x packed per partition, tile-major: block lt = [DC, n+PAD] per
    # partition, cols 2..3 = halo, 4.. = data (tap k reads cols 2+k..)
    xt = nc.dram_tensor("xt", [P, _XBLK[-1]], bf16, kind="ExternalInput").ap()
    # weights packed per partition: wt[p, ec*1024 + dc*128 + j]
    #   = w_pw[ec*128+j, dc*128+p]
    wt = nc.dram_tensor("wt", [P, EC * DC * P], bf16,
                        kind="ExternalInput").ap()
    # per-channel params fp32, cols: w_dw[0..2], b_dw, b_pw
    pp = nc.dram_tensor("pp", [D, 5], f32, kind="ExternalInput").ap()
    # output packed per partition, tile-major: block lt = [EC, n]
    ot = nc.dram_tensor("ot", [P, EC * LC], bf16, kind="ExternalOutput").ap()

    pp_r = pp.rearrange("(o p) c -> p o c", p=P)    # [128, DC, 5]

    with tile.TileContext(nc) as tc:
        with (
            tc.tile_pool(name="wpool", bufs=1) as wpool,
            tc.tile_pool(name="ppool", bufs=1) as ppool,
            tc.tile_pool(name="dpool", bufs=1) as dpool,
            tc.tile_pool(name="xpool5", bufs=3) as xpool5,
            tc.tile_pool(name="xpool2", bufs=2) as xpool2,
            tc.tile_pool(name="ypool", bufs=18) as ypool,
            tc.tile_pool(name="tpool", bufs=6) as tpool,
            tc.tile_pool(name="opool5", bufs=2) as opool5,
            tc.tile_pool(name="opool2", bufs=2) as opool2,
            tc.tile_pool(name="psum", bufs=8, space="PSUM") as psum_pool,
        ):
            p_sb = ppool.tile([P, DC, 5], f32)
            w_sb0 = wpool.tile([P, 1 * DC * P], bf16, name="w_sb0")
            w_sb13 = wpool.tile([P, 3 * DC * P], bf16, name="w_sb13")
            w_sb47 = wpool.tile([P, 4 * DC * P], bf16, name="w_sb47")
            dummy_rhs = dpool.tile([P, 512], bf16, name="dummy_rhs")
            dummy_act = dpool.tile([P, 8], bf16, name="dummy_act")
            dummy_ps = psum_pool.tile([P, 512], f32, tag="acc",
                                      name="acc")

            def w_ap(ec, dc):
                if ec == 0:
                    return w_sb0[:, dc * P:(dc + 1) * P]
                if ec < 4:
                    return w_sb13[:, (ec - 1) * DC * P + dc * P:][:, :P]
                return w_sb47[:, (ec - 4) * DC * P + dc * P:][:, :P]

            def x_load(lt):
                """one contiguous DMA (SP queue) for l-tile lt"""
                n = LTS[lt]
                pool = xpool5 if n == 512 else xpool2
                xs = pool.tile([P, DC, n + PAD], bf16, tag=f"x{n}",
                               name="xs")
                nc.sync.dma_start(
                    xs[:],
                    xt[:, _XBLK[lt]:_XBLK[lt + 1]].rearrange(
                        "p (o c) -> p o c", c=n + PAD))
                return xs

            def dw_conv(lt, xs):
                """depthwise conv: tap0 on ACT, taps 1+2 in-place on DVE;
                on GPS_CHUNKS tap1 rides DVE-TS + GpSimd-TT instead"""
                n = LTS[lt]
                ys = []
                for dc in range(DC):
                    y = ypool.tile([P, 512], bf16, tag="y", name="y")[:, :n]
                    gps = dc in GPS_CHUNKS
                    nc.scalar.activation(
                        y[:], xs[:, dc, 2:2 + n],
                        mybir.ActivationFunctionType.Identity,
                        bias=p_sb[:, dc, 3:4], scale=p_sb[:, dc, 0:1])
                    if gps:
                        t1 = tpool.tile([P, 512], bf16, tag="t1",
                                        name="t1")[:, :n]
                        nc.scalar.activation(
                            t1[:], xs[:, dc, 3:3 + n],
                            mybir.ActivationFunctionType.Identity,
                            bias=0.0, scale=p_sb[:, dc, 1:2])
                    if not gps:
                        nc.vector.scalar_tensor_tensor(
                            y[:], xs[:, dc, 3:3 + n], p_sb[:, dc, 1:2], y[:],
                            op0=mybir.AluOpType.mult,
                            op1=mybir.AluOpType.add)
                    nc.vector.scalar_tensor_tensor(
                        y[:], xs[:, dc, 4:4 + n], p_sb[:, dc, 2:3], y[:],
                        op0=mybir.AluOpType.mult, op1=mybir.AluOpType.add)
                    if gps:
                        nc.gpsimd.tensor_tensor(
                            y[:], y[:], t1[:], op=mybir.AluOpType.add)
                    ys.append(y)
                return ys

            def pointwise(lt, ys, o_sb, ecs):
                """o_sb[:, ec, :] = w_pw[ec] @ y + b_pw[ec] for e-chunks"""
                n = LTS[lt]
                for ec in ecs:
                    acc = psum_pool.tile([P, 512], f32, tag="acc",
                                         name="acc")[:, :n]
                    for dc in range(DC):
                        nc.tensor.matmul(
                            acc[:],
                            lhsT=w_ap(ec, dc),
                            rhs=ys[dc][:],
                            start=(dc == 0), stop=(dc == DC - 1))
                    nc.scalar.activation(
                        o_sb[:, ec, :], acc[:],
                        mybir.ActivationFunctionType.Identity,
                        bias=p_sb[:, ec, 4:5], scale=1.0)

            def o_tile(lt):
                n = LTS[lt]
                pool = opool5 if n == 512 else opool2
                return pool.tile([P, EC, n], bf16, tag=f"o{n}", name="o_sb")

            def store(lt, o_sb):
                n = LTS[lt]
                s = _LT_OFF[lt]
                # alternate queues so no single DGE ring backs up
                eng = nc.gpsimd if lt % 2 == 0 else nc.sync
                eng.dma_start(
                    ot[:, EC * s:EC * (s + n)].rearrange(
                        "p (e c) -> p e c", c=n),
                    o_sb[:])

            # --- emission (guides per-queue FIFO order) -----------------
            # warm-up: DVE memsets a junk rhs, ACT preloads its table, the
            # PE chews dummy matmuls so HAM un-throttles during the DMA wait
            nc.vector.memset(dummy_rhs[:], 0.0)
            nc.scalar.activation(
                dummy_act[:], dummy_rhs[:, 0:8],
                mybir.ActivationFunctionType.Identity, bias=0.0, scale=1.0)

            # x rides the SP HWDGE queue; params + weights ride the
            # (otherwise idle early) GpSimd SWDGE queue
            xs0 = x_load(0)
            nc.gpsimd.dma_start(p_sb[:], pp_r[:])
            nc.gpsimd.dma_start(w_sb0[:], wt[:, 0:DC * P])

            for _ in range(NDUMMY):
                nc.tensor.matmul(dummy_ps[:], lhsT=dummy_rhs[:, 0:P],
                                 rhs=dummy_rhs[:], start=True, stop=True)

            nc.gpsimd.dma_start(w_sb13[:], wt[:, DC * P:4 * DC * P])
            xs1 = x_load(1)
            ys0 = dw_conv(0, xs0)
            nc.gpsimd.dma_start(w_sb47[:], wt[:, 4 * DC * P:8 * DC * P])
            xs2 = x_load(2)
            ys1 = dw_conv(1, xs1)
            # pointwise(t) is emitted AFTER dw_conv(t+1): the evacuations
            # must sit BEHIND the next tile's conv taps in the ACT FIFO,
            # else ACT blocks on matmul-completion waits and starves DVE
            o0 = o_tile(0)
            pointwise(0, ys0, o0, range(EC))
            store(0, o0)

            xs3 = x_load(3)
            ys2 = dw_conv(2, xs2)
            o1 = o_tile(1)
            pointwise(1, ys1, o1, range(EC))
            store(1, o1)

            xs4 = x_load(4)
            ys3 = dw_conv(3, xs3)
            o2 = o_tile(2)
            pointwise(2, ys2, o2, range(EC))
            store(2, o2)

            ys4 = dw_conv(4, xs4)
            o3 = o_tile(3)
            pointwise(3, ys3, o3, range(EC))
            store(3, o3)

            # last tile: store per ec-pair, spread across the three DMA
            # queues so the final drain is short
            n4 = LTS[4]
            s4 = _LT_OFF[4]
            o4 = o_tile(4)
            last_q = [nc.sync, nc.gpsimd, nc.scalar, nc.sync]
            for i in range(4):
                pointwise(4, ys4, o4, range(2 * i, 2 * i + 2))
                dst = ot[:, EC * s4 + 2 * i * n4:EC * s4 + (2 * i + 2) * n4]
                last_q[i].dma_start(
                    dst.rearrange("p (e c) -> p e c", c=n4),
                    o4[:, 2 * i:2 * i + 2, :])

    nc.compile()  # bacc: legalizes multi-sem waits for TRN2 codegen
    return nc


def _shard_inputs(x, w_dw, b_dw, w_pw, b_pw):
    import ml_dtypes
    bf = ml_dtypes.bfloat16
    # wt[p, ec*1024 + dc*128 + j] = w_pw[ec*128+j, dc*128+p]
    wt = np.ascontiguousarray(
        w_pw.reshape(EC, P, DC, P).transpose(3, 0, 2, 1).reshape(P, -1)
    ).astype(bf)
    pp = np.ascontiguousarray(
        np.stack([w_dw[:, 0], w_dw[:, 1], w_dw[:, 2], b_dw, b_pw], axis=1),
        dtype=np.float32)                                        # (D, 5)
    in_maps = []
    for c in range(NCORES):
        b, half = divmod(c, 2)
        l0 = half * LC
        # xpad[d, t]: t 0..1 junk, 2..3 halo (x[l0-2], x[l0-1]), 4.. data
        xpad = np.zeros((D, LC + PAD), dtype=bf)
        lo = max(l0 - 2, 0)
        xpad[:, PAD - (l0 - lo):] = x[b, lo:l0 + LC, :].T.astype(bf)
        xtc = np.empty((P, _XBLK[-1]), dtype=bf)
        for lt, n in enumerate(LTS):
            s = _LT_OFF[lt]
            blk = xpad[:, s:s + n + PAD].reshape(DC, P, n + PAD)
            xtc[:, _XBLK[lt]:_XBLK[lt + 1]] = \
                blk.transpose(1, 0, 2).reshape(P, -1)
        in_maps.append({"xt": xtc, "wt": wt, "pp": pp})
    return in_maps


def kernel(x, w_dw, b_dw, w_pw, b_pw):
    assert x.shape == (B, L, D) and w_dw.shape == (D, KSZ)
    global _CACHED_NC
    if _CACHED_NC is None:
        _CACHED_NC = _build_nc()
    in_maps = _shard_inputs(np.asarray(x, dtype=np.float32),
                            np.asarray(w_dw), np.asarray(b_dw),
                            np.asarray(w_pw), np.asarray(b_pw))
    results = run_bass_kernel_spmd(
        _CACHED_NC, in_maps, list(range(NCORES))).results
    out = np.empty((B, L, D), dtype=np.float32)
    for c in range(NCORES):
        b, half = divmod(c, 2)
        l0 = half * LC
        o = results[c]["ot"]
        for lt, n in enumerate(LTS):
            s = _LT_OFF[lt]
            blk = o[:, EC * s:EC * (s + n)].reshape(P, EC, n)
            out[b, l0 + s:l0 + s + n, :] = \
                blk.transpose(2, 1, 0).reshape(n, D).astype(np.float32)
    return out
